# revision 23
# baseline (speedup 1.0000x reference)
"""Trainium2 Bass kernel for the Alignment-vector problem (v3).

Per batch b (256 total, 32/core across 8 cores):
  q = query * matrix; attn[s,l] = context . q; leaky_relu(0.1);
  l2norm over l; softmax(attn.T * smooth) over s; wc = soft @ context;
  l2norm over d; sim = (query - wc)^2; out = l2norm(sim @ W.T) over S.

v3 design:
  - Softmax denominator cancels in the wc l2norm (only exp numerator kept);
    rsqrt = exp(-0.5*ln(x)) keeps one ACT table set resident.
  - attn + 36x36 Gram matrix from ONE matmul per (batch, chunk): moving
    [qq_c | cT_c] (128x164), stationary cT_c. ||wc||^2 = e^T G e per l.
  - Two-band partition layout: within a 4-batch group, batches 0,1 sit at
    partitions 0-35 and batches 2,3 at 64-99 (both legal matmul tile
    positions), so each small DVE/ACT op covers two batches; group-level
    ops cover all four. This amortizes the ~350cyc ACT / ~150cyc DVE
    fixed costs that dominated earlier versions.
  - Software-pipelined emission: per-engine queues are in-order, so the
    group pipeline is emitted with a 5-phase skew (attn of group g lands
    behind wc of g-3 and out of g-4 in the PE queue). Every instruction's
    inputs are produced phases earlier - no intra-group engine stalls.
  - qq = q*m in place over the m region (DVE); sim = d*d on GPSIMD (the
    only op where GPSIMD's ~0.5 elem/ns/lane beats keeping DVE loaded).
  - NOTE: vector.tensor_tensor_reduce faults on this HW/runtime; free-dim
    reductions go through ACT accum instead.
  - DMA in 4-batch ~1MB slabs; output stored bf16.
"""

import sys

for _p in ("/opt/trn_rl_repo", "/opt/pypackages"):
    if _p not in sys.path:
        sys.path.append(_p)

import numpy as np

N_CORES = 8
B, Lq, Ls, D, S = 256, 128, 36, 1024, 256
BPC = B // N_CORES  # 32
DC = D // 128  # 8
NB = 4  # batches per group == per DMA slab
NGRP = BPC // NB  # 8

_CACHE = {}


def _build(smooth: float, opts=None):
    import concourse.bacc as bacc
    import concourse.tile as tile
    from concourse import mybir

    opts = opts or {}
    QS_BUFS = opts.get("qs_bufs", 5)
    CN_BUFS = opts.get("cn_bufs", 5)
    SM_BUFS = opts.get("sm_bufs", 3)
    D_BUFS = opts.get("d_bufs", 2)
    SIM_BUFS = opts.get("sim_bufs", 2)
    OS_BUFS = opts.get("os_bufs", 2)
    SIM_GP = opts.get("sim_gp", 1)
    QQ_GP = opts.get("qq_gp", 0)
    SCALE_ACT = opts.get("scale_act", 0)
    CN_SCALAR = opts.get("cn_scalar", 1)  # cn loads on the ACT HWDGE ring
    QP_GP_DMA = opts.get("qp_gp_dma", 0)  # 0=sync 1=scalar 2=gpsimd ring for qp
    # sim-only: zero the dead band rows (36-64, 100-128) of PSUM tiles that
    # full-width ops read, so CoreSim's uninitialized-read checker is happy.
    # The dead rows never influence outputs; HW builds omit this.
    SIM_SAFE = opts.get("sim_safe", 0)
    SPLIT_G0 = opts.get("split_g0", 0)  # finer first-group loads+qq (ramp cut)
    # qq(t) emitted at the HEAD of the DVE step queue instead of the tail, so
    # attn32(t) (last in the PE step) never waits on it: the DVE tail
    # (d8 -> en -> max -> qq) otherwise lands at ~12us and gates the period.
    QQ_FIRST = opts.get("qq_first", 0)
    AG_BUFS = opts.get("ag_bufs", 2)  # 1 frees a PSUM bank for wc_bufs=3
    QP_RAMP_SCALAR = opts.get("qp_ramp_scalar", 0)  # qp via ACT ring for ramp groups
    RAMP_GP_LOADS = opts.get("ramp_gp_loads", 0)  # slabs 1-3 via idle GPSIMD ring
    SS_STT = opts.get("ss_stt", 0)  # ss via DVE scalar_tensor_tensor accum

    f32 = mybir.dt.float32
    bf16 = mybir.dt.bfloat16
    A = mybir.ActivationFunctionType
    Op = mybir.AluOpType

    nc = bacc.Bacc("TRN2", target_bir_lowering=False, debug=False)
    # [g, p, j, c, l] = query[b, l, c*128+p], b = g*NB+j
    qp = nc.declare_dram_parameter("qp", [NGRP, 128, NB, DC, Lq], bf16, isOutput=False)
    # [g, p, j, c, 0:128] = matrix[...]; [g, p, j, c, 128:164] = context[b, s, c*128+p]
    mixp = nc.declare_dram_parameter("mixp", [NGRP, 128, NB, DC, 164], bf16, isOutput=False)
    # [g, s, i, d] = context[g*NB + i, s, d]        (band 0: batches 0,1)
    cn0 = nc.declare_dram_parameter("cn0", [NGRP, Ls, 2, D], bf16, isOutput=False)
    # [g, s, i, d] = context[g*NB + 2 + i, s, d]    (band 1: batches 2,3)
    cn1 = nc.declare_dram_parameter("cn1", [NGRP, Ls, 2, D], bf16, isOutput=False)
    wT = nc.declare_dram_parameter("wT", [128, DC, S], bf16, isOutput=False)
    out = nc.declare_dram_parameter("out", [NGRP, Lq, NB, S], bf16, isOutput=True)

    inv_smooth_sq = float(1.0 / (smooth * smooth))

    with tile.TileContext(nc) as tc:
        with (
            tc.tile_pool(name="consts", bufs=1) as consts,
            tc.tile_pool(name="qs", bufs=QS_BUFS) as qs,
            tc.tile_pool(name="cns", bufs=CN_BUFS) as cns,
            tc.tile_pool(name="small", bufs=SM_BUFS) as small,
            tc.tile_pool(name="dp", bufs=D_BUFS) as dp,
            tc.tile_pool(name="simp", bufs=SIM_BUFS) as simp,
            tc.tile_pool(name="osp", bufs=OS_BUFS) as osp,
            tc.tile_pool(name="ps_ag", bufs=AG_BUFS, space="PSUM") as ps_ag,
            tc.tile_pool(name="ps_mid", bufs=opts.get("mid_bufs", 2), space="PSUM") as ps_mid,
            tc.tile_pool(name="ps_wc", bufs=opts.get("wc_bufs", 2), space="PSUM") as ps_wc,
            tc.tile_pool(name="ps_o", bufs=opts.get("o_bufs", 2), space="PSUM") as ps_o,
        ):
            from concourse.hw_specs import get_activation_tables

            set_names = list(get_activation_tables(nc.m.arch).keys())
            nc.scalar.add_instruction(
                mybir.InstLoadActFuncSet(
                    name=nc.get_next_instruction_name(),
                    act_func_set_id=set_names.index("natural_log_exp_and_others"),
                    ins=[],
                    outs=[],
                )
            )

            w_s = consts.tile([128, DC, S], bf16)
            nc.scalar.dma_start(out=w_s, in_=wT[:])
            ones_col = consts.tile([128, 1], bf16)
            nc.vector.memset(ones_col, 1.0)
            ones_row = consts.tile([128, Ls], bf16)
            nc.vector.memset(ones_row, 1.0)

            st = {}  # per-group state

            # Warm the PE clock during the initial DMA ramp: HAM throttles the
            # PE to 1.2 GHz until it sees ~3.4us of sustained activity, and the
            # first real matmuls otherwise run cold. These dummies sit in the
            # PE queue ahead of attn(0), fully inside the first-load shadow.
            N_WARM = opts.get("n_warm", 0)
            if N_WARM:
                warm_p = ps_o.tile([1, S], f32, tag="o", name="warm_p")
                for _ in range(N_WARM):
                    nc.tensor.matmul(warm_p, ones_col, w_s[:, 0], start=True, stop=True)

            def pband(bb):
                return (bb // 2) * 64, bb % 2  # partition base, slot

            def sim_zero(t):
                if SIM_SAFE:
                    nc.vector.memset(t[Ls:64], 0.0)
                    nc.vector.memset(t[64 + Ls : 128], 0.0)

            def p0_load(g):
                # loads split in batch-pair halves so the first attn matmuls
                # only wait on half a slab (cuts the pipeline ramp)
                qp_t = qs.tile([128, NB, DC, Lq], bf16, tag="qp", name="qp_t")
                mix_t = qs.tile([128, NB, DC, 164], bf16, tag="mix", name="mix_t")
                cn_t = cns.tile([128, 2, D], bf16, tag="cn", name="cn_t")
                if g == 0 and SPLIT_G0:
                    nc.sync.dma_start(out=mix_t[:, 0:2], in_=mixp[g, :, 0:2])
                    nc.sync.dma_start(out=qp_t[:, 0:2], in_=qp[g, :, 0:2])
                    nc.sync.dma_start(out=mix_t[:, 2:4], in_=mixp[g, :, 2:4])
                    nc.sync.dma_start(out=qp_t[:, 2:4], in_=qp[g, :, 2:4])
                else:
                    qp_eng = (
                        nc.gpsimd if QP_GP_DMA == 2
                        else (nc.scalar if QP_GP_DMA == 1 else nc.sync)
                    )
                    if QP_RAMP_SCALAR and g < QS_BUFS - 1:
                        qp_eng = nc.scalar
                    mix_eng = nc.sync
                    if RAMP_GP_LOADS and 1 <= g <= 3:
                        qp_eng = mix_eng = nc.gpsimd
                    qp_eng.dma_start(out=qp_t, in_=qp[g])
                    mix_eng.dma_start(out=mix_t, in_=mixp[g])
                cn_eng = nc.scalar if CN_SCALAR else nc.sync
                cn_eng.dma_start(out=cn_t[0:Ls], in_=cn0[g])
                cn_eng.dma_start(out=cn_t[64 : 64 + Ls], in_=cn1[g])
                st[g] = {"qp": qp_t, "mix": mix_t, "cn": cn_t}

            def p1_qq(g):
                s = st[g]
                qp_t, mix_t = s["qp"], s["mix"]
                qq_eng = nc.gpsimd if QQ_GP else nc.vector
                if g == 0 and SPLIT_G0:
                    qq_eng.tensor_mul(
                        mix_t[:, 0:2, :, 0:128], qp_t[:, 0:2], mix_t[:, 0:2, :, 0:128]
                    )
                    qq_eng.tensor_mul(
                        mix_t[:, 2:4, :, 0:128], qp_t[:, 2:4], mix_t[:, 2:4, :, 0:128]
                    )
                else:
                    qq_eng.tensor_mul(
                        mix_t[:, :, :, 0:128], qp_t, mix_t[:, :, :, 0:128]
                    )

            def p1_attn(g):
                s = st[g]
                mix_t = s["mix"]
                if not QQ_FIRST:
                    p1_qq(g)
                ag_p = ps_ag.tile([128, 2, 164], f32, tag="ag", name="ag_p")
                sim_zero(ag_p)
                for bb in range(NB):
                    pb, sl = pband(bb)
                    for c in range(DC):
                        nc.tensor.matmul(
                            ag_p[pb : pb + Ls, sl],
                            mix_t[:, bb, c, 128:164],
                            mix_t[:, bb, c],
                            start=(c == 0),
                            stop=(c == DC - 1),
                        )
                s["ag"] = ag_p

            def p2_softmax(g):
                s = st[g]
                ag_p = s["ag"]
                y0_t = small.tile([128, 2, Lq], f32, tag="y0", name="y0_t")
                nc.scalar.activation(out=y0_t, in_=ag_p[:, :, 0:128], func=A.Copy, scale=0.1)
                y_t = small.tile([128, 2, Lq], f32, tag="y", name="y_t")
                nc.vector.tensor_max(y_t, y0_t, ag_p[:, :, 0:128])
                G_s = small.tile([128, 2, Ls], bf16, tag="G", name="G_s")
                nc.vector.tensor_copy(G_s, ag_p[:, :, 128:164])
                ss_t = small.tile([128, 2], f32, tag="ss", name="ss_t")
                sq_t = small.tile([128, Lq], f32, tag="sq", name="sq_t")
                for i in range(2):
                    if SS_STT:
                        nc.vector.scalar_tensor_tensor(
                            sq_t, y_t[:, i], 1.0, y_t[:, i],
                            op0=Op.bypass, op1=Op.mult,
                            accum_out=ss_t[:, i : i + 1],
                        )
                    else:
                        nc.scalar.activation(
                            out=sq_t, in_=y_t[:, i], func=A.Square,
                            accum_out=ss_t[:, i : i + 1],
                        )
                lnss_t = small.tile([128, 2], f32, tag="lnss", name="lnss_t")
                nc.scalar.activation(out=lnss_t, in_=ss_t, func=A.Ln, scale=inv_smooth_sq)
                r9_t = small.tile([128, 2], f32, tag="r9", name="r9_t")
                nc.scalar.activation(out=r9_t, in_=lnss_t, func=A.Exp, scale=-0.5)
                e_t = small.tile([128, 2, Lq], bf16, tag="e", name="e_t")
                for i in range(2):
                    nc.scalar.activation(
                        out=e_t[:, i], in_=y_t[:, i], func=A.Exp,
                        scale=r9_t[:, i : i + 1],
                    )
                s["G"], s["e"] = G_s, e_t

            def p3_norm(g):
                s = st[g]
                G_s, e_t = s["G"], s["e"]
                h_p = ps_mid.tile([128, 2, Lq], f32, tag="mid", name="h_p")
                sim_zero(h_p)
                for bb in range(NB):
                    pb, sl = pband(bb)
                    nc.tensor.matmul(
                        h_p[pb : pb + Ls, sl],
                        G_s[pb : pb + Ls, sl],
                        e_t[pb : pb + Ls, sl],
                        start=True,
                        stop=True,
                    )
                eh_t = small.tile([128, 2, Lq], bf16, tag="eh", name="eh_t")
                nc.vector.tensor_mul(eh_t, e_t, h_p)
                ssl_p = ps_mid.tile([128, 2 * Lq], f32, tag="mid", name="ssl_p")
                if SIM_SAFE:
                    nc.vector.memset(ssl_p[1:64], 0.0)
                    nc.vector.memset(ssl_p[65:128], 0.0)
                for band in range(2):
                    pb = band * 64
                    nc.tensor.matmul(
                        ssl_p[pb : pb + 1],
                        ones_col[pb : pb + Ls],
                        eh_t[pb : pb + Ls].rearrange("p a b -> p (a b)"),
                        start=True,
                        stop=True,
                    )
                lnl_t = small.tile([128, 2 * Lq], f32, tag="lnl", name="lnl_t")
                nc.scalar.activation(out=lnl_t, in_=ssl_p, func=A.Ln)
                k_t = small.tile([128, 2 * Lq], bf16, tag="k", name="k_t")
                nc.scalar.activation(out=k_t, in_=lnl_t, func=A.Exp, scale=-0.5)
                s["k"] = k_t

            def p3b_kb(g):
                s = st[g]
                e_t, k_t = s["e"], s["k"]
                kb_p = ps_mid.tile([128, 2, Lq], f32, tag="mid", name="kb_p")
                sim_zero(kb_p)
                for band in range(2):
                    pb = band * 64
                    nc.tensor.matmul(
                        kb_p[pb : pb + Ls].rearrange("p a b -> p (a b)"),
                        ones_row[pb : pb + 1],
                        k_t[pb : pb + 1],
                        start=True,
                        stop=True,
                    )
                en_t = small.tile([128, 2, Lq], bf16, tag="en", name="en_t")
                nc.vector.tensor_mul(en_t, e_t, kb_p)
                s["en"] = en_t

            def p4_wc(g):
                s = st[g]
                qp_t, cn_t, en_t = s["qp"], s["cn"], s["en"]
                sim_t = simp.tile([128, NB, DC, Lq], bf16, tag="sim", name="sim_t")
                for bb in range(NB):
                    pb, sl = pband(bb)
                    for h in range(2):
                        wc_p = ps_wc.tile([128, 4, Lq], f32, tag="wc", name="wc_p")
                        for ci in range(4):
                            c = h * 4 + ci
                            nc.tensor.matmul(
                                wc_p[:, ci],
                                cn_t[pb : pb + Ls, sl, c * 128 : (c + 1) * 128],
                                en_t[pb : pb + Ls, sl],
                                start=True,
                                stop=True,
                            )
                        hsl = slice(h * 4, (h + 1) * 4)
                        d_t = dp.tile([128, 4, Lq], bf16, tag="d", name="d_t")
                        nc.vector.tensor_sub(d_t, qp_t[:, bb, hsl], wc_p)
                        sim_eng = nc.gpsimd if SIM_GP else nc.vector
                        sim_eng.tensor_mul(sim_t[:, bb, hsl], d_t, d_t)
                s["sim"] = sim_t

            def p5_out(g):
                s = st[g]
                sim_t = s["sim"]
                ss3_t = small.tile([Lq, NB], f32, tag="ss3", name="ss3_t")
                sq3_t = small.tile([Lq, S], f32, tag="sq3", name="sq3_t")
                ln3_t = small.tile([Lq, NB], f32, tag="ln3", name="ln3_t")
                r3_t = small.tile([Lq, NB], f32, tag="r3", name="r3_t")
                o_s = osp.tile([Lq, NB, S], bf16, tag="os", name="o_s")
                # ln3/r3/scale are emitted per batch PAIR so the scale that
                # frees a ps_o slot never sits behind a dependent sq3 on the
                # in-order ACT queue (ps_o has only 2 bufs).
                for half in range(2):
                    o_ps = []
                    for bb in range(2 * half, 2 * half + 2):
                        o_p = ps_o.tile([Lq, S], f32, tag="o", name="o_p")
                        o_ps.append(o_p)
                        for c in range(DC):
                            nc.tensor.matmul(
                                o_p,
                                sim_t[:, bb, c],
                                w_s[:, c],
                                start=(c == 0),
                                stop=(c == DC - 1),
                            )
                        nc.scalar.activation(
                            out=sq3_t, in_=o_p, func=A.Square,
                            accum_out=ss3_t[:, bb : bb + 1],
                        )
                    hs = slice(2 * half, 2 * half + 2)
                    nc.scalar.activation(out=ln3_t[:, hs], in_=ss3_t[:, hs], func=A.Ln)
                    nc.scalar.activation(
                        out=r3_t[:, hs], in_=ln3_t[:, hs], func=A.Exp, scale=-0.5
                    )
                    for j, bb in enumerate(range(2 * half, 2 * half + 2)):
                        if SCALE_ACT:
                            nc.scalar.activation(
                                out=o_s[:, bb], in_=o_ps[j], func=A.Copy,
                                scale=r3_t[:, bb : bb + 1],
                            )
                        else:
                            nc.vector.tensor_scalar_mul(
                                o_s[:, bb], o_ps[j], r3_t[:, bb : bb + 1]
                            )
                nc.sync.dma_start(out=out[g], in_=o_s)
                del st[g]

            # emission order per step: P3's producers (h/eh/ssl/lnl/k) go
            # BEFORE P4 so DVE/ACT reach them early; kb/en go AFTER P4 so the
            # PE reaches kb only after wc32, when k is already computed.
            sched = [(p5_out, 4), (p3_norm, 2), (p4_wc, 3), (p3b_kb, 2),
                     (p2_softmax, 1), (p1_attn, 0)]
            if opts.get("p3_split", 1) == 0:
                sched = [(p5_out, 4), (p4_wc, 3), (p3_norm, 2), (p3b_kb, 2),
                         (p2_softmax, 1), (p1_attn, 0)]
            if QQ_FIRST:
                sched = [(p1_qq, 0)] + sched
            # keep-warm dummies at the top of drain steps: the tail has real PE
            # stalls (d-paced wc) that otherwise re-throttle the HAM clock and
            # the last groups' out-matmuls run at 1.2 GHz (~223ns vs 116ns).
            TAIL_WARM = opts.get("tail_warm", 0)

            def tail_warm_burst():
                warm_p = ps_o.tile([1, S], f32, tag="o", name="warm_p")
                for _ in range(TAIL_WARM):
                    nc.tensor.matmul(warm_p, ones_col, w_s[:, 0], start=True, stop=True)

            p0_load(0)
            for t in range(NGRP + 4 + 1):
                if TAIL_WARM and t >= NGRP:
                    tail_warm_burst()
                if t + 1 < NGRP:
                    p0_load(t + 1)
                for ph, lag in sched:
                    gk = t - lag
                    if 0 <= gk < NGRP:
                        ph(gk)

    nc.compile()
    return nc


def _prep_inputs(query, context, matrix, smooth, W, b):
    import ml_dtypes

    bf16 = ml_dtypes.bfloat16
    qT = query.reshape(B, Lq, DC, 128).transpose(0, 3, 2, 1).astype(bf16)
    mT = matrix.reshape(B, Lq, DC, 128).transpose(0, 3, 2, 1).astype(bf16)
    cT = context.reshape(B, Ls, DC, 128).transpose(0, 3, 2, 1).astype(bf16)
    mix = np.concatenate([mT, cT], axis=3)  # [B, 128, DC, 164]
    cN = np.ascontiguousarray(context).astype(bf16)  # [B, 36, 1024]
    wTh = W.reshape(S, DC, 128).transpose(2, 1, 0).astype(bf16)

    in_maps = []
    for i in range(N_CORES):
        sl = slice(i * BPC, (i + 1) * BPC)
        qpc = qT[sl].reshape(NGRP, NB, 128, DC, Lq).transpose(0, 2, 1, 3, 4)
        mixc = mix[sl].reshape(NGRP, NB, 128, DC, 164).transpose(0, 2, 1, 3, 4)
        cnc = cN[sl].reshape(NGRP, NB, Ls, D)
        cn0c = cnc[:, 0:2].transpose(0, 2, 1, 3)  # [NGRP, 36, 2, D]
        cn1c = cnc[:, 2:4].transpose(0, 2, 1, 3)
        in_maps.append(
            {
                "qp": np.ascontiguousarray(qpc),
                "mixp": np.ascontiguousarray(mixc),
                "cn0": np.ascontiguousarray(cn0c),
                "cn1": np.ascontiguousarray(cn1c),
                "wT": wTh,
            }
        )
    return in_maps


def _run(query, context, matrix, smooth, W, b, trace=False, opts=None):
    from concourse.bass_utils import run_bass_kernel_spmd

    smooth_f = float(smooth)
    key = (smooth_f, str(sorted((opts or {}).items())))
    if key not in _CACHE:
        _CACHE[key] = _build(smooth_f, opts)
    nc = _CACHE[key]

    in_maps = _prep_inputs(query, context, matrix, smooth_f, W, b)
    res = run_bass_kernel_spmd(nc, in_maps, core_ids=list(range(N_CORES)), trace=trace)
    parts = []
    for i in range(N_CORES):
        o = np.asarray(res.results[i]["out"], dtype=np.float32)
        parts.append(o.transpose(0, 2, 1, 3).reshape(BPC, Lq, S))
    return np.concatenate(parts, axis=0), res


def kernel(query, context, matrix, smooth, W, b):
    query = np.asarray(query, dtype=np.float32)
    context = np.asarray(context, dtype=np.float32)
    matrix = np.asarray(matrix, dtype=np.float32)
    W = np.asarray(W, dtype=np.float32)
    b = np.asarray(b, dtype=np.float32)
    out, _ = _run(query, context, matrix, smooth, W, b, trace=False)
    return out


def kernel_profiled(query, context, matrix, smooth, W, b, reps=3, opts=None):
    out, res = _run(query, context, matrix, smooth, W, b, trace=True, opts=opts)
    times = [res.exec_time_ns]
    for _ in range(reps - 1):
        _, r2 = _run(query, context, matrix, smooth, W, b, trace=True, opts=opts)
        times.append(r2.exec_time_ns)
    res.all_times = times
    return out, res


# revision 24
# speedup vs baseline: 1.0748x; 1.0748x over previous
"""Trainium2 Bass kernel for the Alignment-vector problem (v3).

Per batch b (256 total, 32/core across 8 cores):
  q = query * matrix; attn[s,l] = context . q; leaky_relu(0.1);
  l2norm over l; softmax(attn.T * smooth) over s; wc = soft @ context;
  l2norm over d; sim = (query - wc)^2; out = l2norm(sim @ W.T) over S.

v3 design:
  - Softmax denominator cancels in the wc l2norm (only exp numerator kept);
    rsqrt = exp(-0.5*ln(x)) keeps one ACT table set resident.
  - attn + 36x36 Gram matrix from ONE matmul per (batch, chunk): moving
    [qq_c | cT_c] (128x164), stationary cT_c. ||wc||^2 = e^T G e per l.
  - Two-band partition layout: within a 4-batch group, batches 0,1 sit at
    partitions 0-35 and batches 2,3 at 64-99 (both legal matmul tile
    positions), so each small DVE/ACT op covers two batches; group-level
    ops cover all four. This amortizes the ~350cyc ACT / ~150cyc DVE
    fixed costs that dominated earlier versions.
  - Software-pipelined emission: per-engine queues are in-order, so the
    group pipeline is emitted with a 5-phase skew (attn of group g lands
    behind wc of g-3 and out of g-4 in the PE queue). Every instruction's
    inputs are produced phases earlier - no intra-group engine stalls.
  - qq = q*m in place over the m region (DVE); sim = d*d on GPSIMD (the
    only op where GPSIMD's ~0.5 elem/ns/lane beats keeping DVE loaded).
  - NOTE: vector.tensor_tensor_reduce faults on this HW/runtime; free-dim
    reductions go through ACT accum instead.
  - DMA in 4-batch ~1MB slabs; output stored bf16.
"""

import sys

for _p in ("/opt/trn_rl_repo", "/opt/pypackages"):
    if _p not in sys.path:
        sys.path.append(_p)

import numpy as np

N_CORES = 8
B, Lq, Ls, D, S = 256, 128, 36, 1024, 256
BPC = B // N_CORES  # 32
DC = D // 128  # 8
NB = 4  # batches per group == per DMA slab
NGRP = BPC // NB  # 8

_CACHE = {}


def _build(smooth: float, opts=None):
    import concourse.bacc as bacc
    import concourse.tile as tile
    from concourse import mybir

    opts = opts or {}
    QS_BUFS = opts.get("qs_bufs", 5)
    CN_BUFS = opts.get("cn_bufs", 5)
    SM_BUFS = opts.get("sm_bufs", 3)
    # 3 bufs, not 2: with 2, the DVE d-sub chain runs in lockstep with the
    # slower GpSimd sim consumer (d(k+2) waits sim(k), 1055ns vs 658ns/unit),
    # throttling the whole wc phase. A third buffer decouples them: measured
    # 151.4us -> 143.4us. Deeper (4,6) measures slightly worse.
    D_BUFS = opts.get("d_bufs", 3)
    SIM_BUFS = opts.get("sim_bufs", 2)
    OS_BUFS = opts.get("os_bufs", 2)
    SIM_GP = opts.get("sim_gp", 1)
    QQ_GP = opts.get("qq_gp", 0)
    SCALE_ACT = opts.get("scale_act", 0)
    CN_SCALAR = opts.get("cn_scalar", 1)  # cn loads on the ACT HWDGE ring
    QP_GP_DMA = opts.get("qp_gp_dma", 0)  # 0=sync 1=scalar 2=gpsimd ring for qp
    # sim-only: zero the dead band rows (36-64, 100-128) of PSUM tiles that
    # full-width ops read, so CoreSim's uninitialized-read checker is happy.
    # The dead rows never influence outputs; HW builds omit this.
    SIM_SAFE = opts.get("sim_safe", 0)
    SPLIT_G0 = opts.get("split_g0", 0)  # finer first-group loads+qq (ramp cut)
    # qq(t) emitted at the HEAD of the DVE step queue instead of the tail, so
    # attn32(t) (last in the PE step) never waits on it: the DVE tail
    # (d8 -> en -> max -> qq) otherwise lands at ~12us and gates the period.
    QQ_FIRST = opts.get("qq_first", 0)
    AG_BUFS = opts.get("ag_bufs", 2)  # 1 frees a PSUM bank for wc_bufs=3
    QP_RAMP_SCALAR = opts.get("qp_ramp_scalar", 0)  # qp via ACT ring for ramp groups
    RAMP_GP_LOADS = opts.get("ramp_gp_loads", 0)  # slabs 1-3 via idle GPSIMD ring
    SS_STT = opts.get("ss_stt", 0)  # ss via DVE scalar_tensor_tensor accum

    f32 = mybir.dt.float32
    bf16 = mybir.dt.bfloat16
    A = mybir.ActivationFunctionType
    Op = mybir.AluOpType

    nc = bacc.Bacc("TRN2", target_bir_lowering=False, debug=False)
    # [g, p, j, c, l] = query[b, l, c*128+p], b = g*NB+j
    qp = nc.declare_dram_parameter("qp", [NGRP, 128, NB, DC, Lq], bf16, isOutput=False)
    # [g, p, j, c, 0:128] = matrix[...]; [g, p, j, c, 128:164] = context[b, s, c*128+p]
    mixp = nc.declare_dram_parameter("mixp", [NGRP, 128, NB, DC, 164], bf16, isOutput=False)
    # [g, s, i, d] = context[g*NB + i, s, d]        (band 0: batches 0,1)
    cn0 = nc.declare_dram_parameter("cn0", [NGRP, Ls, 2, D], bf16, isOutput=False)
    # [g, s, i, d] = context[g*NB + 2 + i, s, d]    (band 1: batches 2,3)
    cn1 = nc.declare_dram_parameter("cn1", [NGRP, Ls, 2, D], bf16, isOutput=False)
    wT = nc.declare_dram_parameter("wT", [128, DC, S], bf16, isOutput=False)
    out = nc.declare_dram_parameter("out", [NGRP, Lq, NB, S], bf16, isOutput=True)

    inv_smooth_sq = float(1.0 / (smooth * smooth))

    with tile.TileContext(nc) as tc:
        with (
            tc.tile_pool(name="consts", bufs=1) as consts,
            tc.tile_pool(name="qs", bufs=QS_BUFS) as qs,
            tc.tile_pool(name="cns", bufs=CN_BUFS) as cns,
            tc.tile_pool(name="small", bufs=SM_BUFS) as small,
            tc.tile_pool(name="dp", bufs=D_BUFS) as dp,
            tc.tile_pool(name="simp", bufs=SIM_BUFS) as simp,
            tc.tile_pool(name="osp", bufs=OS_BUFS) as osp,
            tc.tile_pool(name="ps_ag", bufs=AG_BUFS, space="PSUM") as ps_ag,
            tc.tile_pool(name="ps_mid", bufs=opts.get("mid_bufs", 2), space="PSUM") as ps_mid,
            tc.tile_pool(name="ps_wc", bufs=opts.get("wc_bufs", 2), space="PSUM") as ps_wc,
            tc.tile_pool(name="ps_o", bufs=opts.get("o_bufs", 2), space="PSUM") as ps_o,
        ):
            from concourse.hw_specs import get_activation_tables

            set_names = list(get_activation_tables(nc.m.arch).keys())
            nc.scalar.add_instruction(
                mybir.InstLoadActFuncSet(
                    name=nc.get_next_instruction_name(),
                    act_func_set_id=set_names.index("natural_log_exp_and_others"),
                    ins=[],
                    outs=[],
                )
            )

            w_s = consts.tile([128, DC, S], bf16)
            nc.scalar.dma_start(out=w_s, in_=wT[:])
            ones_col = consts.tile([128, 1], bf16)
            nc.vector.memset(ones_col, 1.0)
            ones_row = consts.tile([128, Ls], bf16)
            nc.vector.memset(ones_row, 1.0)

            st = {}  # per-group state

            # Warm the PE clock during the initial DMA ramp: HAM throttles the
            # PE to 1.2 GHz until it sees ~3.4us of sustained activity, and the
            # first real matmuls otherwise run cold. These dummies sit in the
            # PE queue ahead of attn(0), fully inside the first-load shadow.
            N_WARM = opts.get("n_warm", 0)
            if N_WARM:
                warm_p = ps_o.tile([1, S], f32, tag="o", name="warm_p")
                for _ in range(N_WARM):
                    nc.tensor.matmul(warm_p, ones_col, w_s[:, 0], start=True, stop=True)

            def pband(bb):
                return (bb // 2) * 64, bb % 2  # partition base, slot

            def sim_zero(t):
                if SIM_SAFE:
                    nc.vector.memset(t[Ls:64], 0.0)
                    nc.vector.memset(t[64 + Ls : 128], 0.0)

            def p0_load(g):
                # loads split in batch-pair halves so the first attn matmuls
                # only wait on half a slab (cuts the pipeline ramp)
                qp_t = qs.tile([128, NB, DC, Lq], bf16, tag="qp", name="qp_t")
                mix_t = qs.tile([128, NB, DC, 164], bf16, tag="mix", name="mix_t")
                cn_t = cns.tile([128, 2, D], bf16, tag="cn", name="cn_t")
                if g == 0 and SPLIT_G0:
                    nc.sync.dma_start(out=mix_t[:, 0:2], in_=mixp[g, :, 0:2])
                    nc.sync.dma_start(out=qp_t[:, 0:2], in_=qp[g, :, 0:2])
                    nc.sync.dma_start(out=mix_t[:, 2:4], in_=mixp[g, :, 2:4])
                    nc.sync.dma_start(out=qp_t[:, 2:4], in_=qp[g, :, 2:4])
                else:
                    qp_eng = (
                        nc.gpsimd if QP_GP_DMA == 2
                        else (nc.scalar if QP_GP_DMA == 1 else nc.sync)
                    )
                    if QP_RAMP_SCALAR and g < QS_BUFS - 1:
                        qp_eng = nc.scalar
                    mix_eng = nc.sync
                    if RAMP_GP_LOADS and 1 <= g <= 3:
                        qp_eng = mix_eng = nc.gpsimd
                    qp_eng.dma_start(out=qp_t, in_=qp[g])
                    mix_eng.dma_start(out=mix_t, in_=mixp[g])
                cn_eng = nc.scalar if CN_SCALAR else nc.sync
                cn_eng.dma_start(out=cn_t[0:Ls], in_=cn0[g])
                cn_eng.dma_start(out=cn_t[64 : 64 + Ls], in_=cn1[g])
                st[g] = {"qp": qp_t, "mix": mix_t, "cn": cn_t}

            def p1_qq(g):
                s = st[g]
                qp_t, mix_t = s["qp"], s["mix"]
                qq_eng = nc.gpsimd if QQ_GP else nc.vector
                if g == 0 and SPLIT_G0:
                    qq_eng.tensor_mul(
                        mix_t[:, 0:2, :, 0:128], qp_t[:, 0:2], mix_t[:, 0:2, :, 0:128]
                    )
                    qq_eng.tensor_mul(
                        mix_t[:, 2:4, :, 0:128], qp_t[:, 2:4], mix_t[:, 2:4, :, 0:128]
                    )
                else:
                    qq_eng.tensor_mul(
                        mix_t[:, :, :, 0:128], qp_t, mix_t[:, :, :, 0:128]
                    )

            def p1_attn(g):
                s = st[g]
                mix_t = s["mix"]
                if not QQ_FIRST:
                    p1_qq(g)
                ag_p = ps_ag.tile([128, 2, 164], f32, tag="ag", name="ag_p")
                sim_zero(ag_p)
                for bb in range(NB):
                    pb, sl = pband(bb)
                    for c in range(DC):
                        nc.tensor.matmul(
                            ag_p[pb : pb + Ls, sl],
                            mix_t[:, bb, c, 128:164],
                            mix_t[:, bb, c],
                            start=(c == 0),
                            stop=(c == DC - 1),
                        )
                s["ag"] = ag_p

            def p2_softmax(g):
                s = st[g]
                ag_p = s["ag"]
                y0_t = small.tile([128, 2, Lq], f32, tag="y0", name="y0_t")
                nc.scalar.activation(out=y0_t, in_=ag_p[:, :, 0:128], func=A.Copy, scale=0.1)
                y_t = small.tile([128, 2, Lq], f32, tag="y", name="y_t")
                nc.vector.tensor_max(y_t, y0_t, ag_p[:, :, 0:128])
                G_s = small.tile([128, 2, Ls], bf16, tag="G", name="G_s")
                nc.vector.tensor_copy(G_s, ag_p[:, :, 128:164])
                ss_t = small.tile([128, 2], f32, tag="ss", name="ss_t")
                sq_t = small.tile([128, Lq], f32, tag="sq", name="sq_t")
                for i in range(2):
                    if SS_STT:
                        nc.vector.scalar_tensor_tensor(
                            sq_t, y_t[:, i], 1.0, y_t[:, i],
                            op0=Op.bypass, op1=Op.mult,
                            accum_out=ss_t[:, i : i + 1],
                        )
                    else:
                        nc.scalar.activation(
                            out=sq_t, in_=y_t[:, i], func=A.Square,
                            accum_out=ss_t[:, i : i + 1],
                        )
                lnss_t = small.tile([128, 2], f32, tag="lnss", name="lnss_t")
                nc.scalar.activation(out=lnss_t, in_=ss_t, func=A.Ln, scale=inv_smooth_sq)
                r9_t = small.tile([128, 2], f32, tag="r9", name="r9_t")
                nc.scalar.activation(out=r9_t, in_=lnss_t, func=A.Exp, scale=-0.5)
                e_t = small.tile([128, 2, Lq], bf16, tag="e", name="e_t")
                for i in range(2):
                    nc.scalar.activation(
                        out=e_t[:, i], in_=y_t[:, i], func=A.Exp,
                        scale=r9_t[:, i : i + 1],
                    )
                s["G"], s["e"] = G_s, e_t

            def p3_norm(g):
                s = st[g]
                G_s, e_t = s["G"], s["e"]
                h_p = ps_mid.tile([128, 2, Lq], f32, tag="mid", name="h_p")
                sim_zero(h_p)
                for bb in range(NB):
                    pb, sl = pband(bb)
                    nc.tensor.matmul(
                        h_p[pb : pb + Ls, sl],
                        G_s[pb : pb + Ls, sl],
                        e_t[pb : pb + Ls, sl],
                        start=True,
                        stop=True,
                    )
                eh_t = small.tile([128, 2, Lq], bf16, tag="eh", name="eh_t")
                nc.vector.tensor_mul(eh_t, e_t, h_p)
                ssl_p = ps_mid.tile([128, 2 * Lq], f32, tag="mid", name="ssl_p")
                if SIM_SAFE:
                    nc.vector.memset(ssl_p[1:64], 0.0)
                    nc.vector.memset(ssl_p[65:128], 0.0)
                for band in range(2):
                    pb = band * 64
                    nc.tensor.matmul(
                        ssl_p[pb : pb + 1],
                        ones_col[pb : pb + Ls],
                        eh_t[pb : pb + Ls].rearrange("p a b -> p (a b)"),
                        start=True,
                        stop=True,
                    )
                lnl_t = small.tile([128, 2 * Lq], f32, tag="lnl", name="lnl_t")
                nc.scalar.activation(out=lnl_t, in_=ssl_p, func=A.Ln)
                k_t = small.tile([128, 2 * Lq], bf16, tag="k", name="k_t")
                nc.scalar.activation(out=k_t, in_=lnl_t, func=A.Exp, scale=-0.5)
                s["k"] = k_t

            def p3b_kb(g):
                s = st[g]
                e_t, k_t = s["e"], s["k"]
                kb_p = ps_mid.tile([128, 2, Lq], f32, tag="mid", name="kb_p")
                sim_zero(kb_p)
                for band in range(2):
                    pb = band * 64
                    nc.tensor.matmul(
                        kb_p[pb : pb + Ls].rearrange("p a b -> p (a b)"),
                        ones_row[pb : pb + 1],
                        k_t[pb : pb + 1],
                        start=True,
                        stop=True,
                    )
                en_t = small.tile([128, 2, Lq], bf16, tag="en", name="en_t")
                nc.vector.tensor_mul(en_t, e_t, kb_p)
                s["en"] = en_t

            def p4_wc(g):
                s = st[g]
                qp_t, cn_t, en_t = s["qp"], s["cn"], s["en"]
                sim_t = simp.tile([128, NB, DC, Lq], bf16, tag="sim", name="sim_t")
                for bb in range(NB):
                    pb, sl = pband(bb)
                    for h in range(2):
                        wc_p = ps_wc.tile([128, 4, Lq], f32, tag="wc", name="wc_p")
                        for ci in range(4):
                            c = h * 4 + ci
                            nc.tensor.matmul(
                                wc_p[:, ci],
                                cn_t[pb : pb + Ls, sl, c * 128 : (c + 1) * 128],
                                en_t[pb : pb + Ls, sl],
                                start=True,
                                stop=True,
                            )
                        hsl = slice(h * 4, (h + 1) * 4)
                        d_t = dp.tile([128, 4, Lq], bf16, tag="d", name="d_t")
                        nc.vector.tensor_sub(d_t, qp_t[:, bb, hsl], wc_p)
                        sim_eng = nc.gpsimd if SIM_GP else nc.vector
                        sim_eng.tensor_mul(sim_t[:, bb, hsl], d_t, d_t)
                s["sim"] = sim_t

            def p5_out(g):
                s = st[g]
                sim_t = s["sim"]
                ss3_t = small.tile([Lq, NB], f32, tag="ss3", name="ss3_t")
                sq3_t = small.tile([Lq, S], f32, tag="sq3", name="sq3_t")
                ln3_t = small.tile([Lq, NB], f32, tag="ln3", name="ln3_t")
                r3_t = small.tile([Lq, NB], f32, tag="r3", name="r3_t")
                o_s = osp.tile([Lq, NB, S], bf16, tag="os", name="o_s")
                # ln3/r3/scale are emitted per batch PAIR so the scale that
                # frees a ps_o slot never sits behind a dependent sq3 on the
                # in-order ACT queue (ps_o has only 2 bufs).
                for half in range(2):
                    o_ps = []
                    for bb in range(2 * half, 2 * half + 2):
                        o_p = ps_o.tile([Lq, S], f32, tag="o", name="o_p")
                        o_ps.append(o_p)
                        for c in range(DC):
                            nc.tensor.matmul(
                                o_p,
                                sim_t[:, bb, c],
                                w_s[:, c],
                                start=(c == 0),
                                stop=(c == DC - 1),
                            )
                        nc.scalar.activation(
                            out=sq3_t, in_=o_p, func=A.Square,
                            accum_out=ss3_t[:, bb : bb + 1],
                        )
                    hs = slice(2 * half, 2 * half + 2)
                    nc.scalar.activation(out=ln3_t[:, hs], in_=ss3_t[:, hs], func=A.Ln)
                    nc.scalar.activation(
                        out=r3_t[:, hs], in_=ln3_t[:, hs], func=A.Exp, scale=-0.5
                    )
                    for j, bb in enumerate(range(2 * half, 2 * half + 2)):
                        if SCALE_ACT:
                            nc.scalar.activation(
                                out=o_s[:, bb], in_=o_ps[j], func=A.Copy,
                                scale=r3_t[:, bb : bb + 1],
                            )
                        else:
                            nc.vector.tensor_scalar_mul(
                                o_s[:, bb], o_ps[j], r3_t[:, bb : bb + 1]
                            )
                nc.sync.dma_start(out=out[g], in_=o_s)
                del st[g]

            # emission order per step: P3's producers (h/eh/ssl/lnl/k) go
            # BEFORE P4 so DVE/ACT reach them early; kb/en go AFTER P4 so the
            # PE reaches kb only after wc32, when k is already computed.
            sched = [(p5_out, 4), (p3_norm, 2), (p4_wc, 3), (p3b_kb, 2),
                     (p2_softmax, 1), (p1_attn, 0)]
            if opts.get("p3_split", 1) == 0:
                sched = [(p5_out, 4), (p4_wc, 3), (p3_norm, 2), (p3b_kb, 2),
                         (p2_softmax, 1), (p1_attn, 0)]
            if QQ_FIRST:
                sched = [(p1_qq, 0)] + sched
            # keep-warm dummies at the top of drain steps: the tail has real PE
            # stalls (d-paced wc) that otherwise re-throttle the HAM clock and
            # the last groups' out-matmuls run at 1.2 GHz (~223ns vs 116ns).
            TAIL_WARM = opts.get("tail_warm", 0)

            def tail_warm_burst():
                warm_p = ps_o.tile([1, S], f32, tag="o", name="warm_p")
                for _ in range(TAIL_WARM):
                    nc.tensor.matmul(warm_p, ones_col, w_s[:, 0], start=True, stop=True)

            p0_load(0)
            for t in range(NGRP + 4 + 1):
                if TAIL_WARM and t >= NGRP:
                    tail_warm_burst()
                if t + 1 < NGRP:
                    p0_load(t + 1)
                for ph, lag in sched:
                    gk = t - lag
                    if 0 <= gk < NGRP:
                        ph(gk)

    nc.compile()
    return nc


def _prep_inputs(query, context, matrix, smooth, W, b):
    import ml_dtypes

    bf16 = ml_dtypes.bfloat16
    qT = query.reshape(B, Lq, DC, 128).transpose(0, 3, 2, 1).astype(bf16)
    mT = matrix.reshape(B, Lq, DC, 128).transpose(0, 3, 2, 1).astype(bf16)
    cT = context.reshape(B, Ls, DC, 128).transpose(0, 3, 2, 1).astype(bf16)
    mix = np.concatenate([mT, cT], axis=3)  # [B, 128, DC, 164]
    cN = np.ascontiguousarray(context).astype(bf16)  # [B, 36, 1024]
    wTh = W.reshape(S, DC, 128).transpose(2, 1, 0).astype(bf16)

    in_maps = []
    for i in range(N_CORES):
        sl = slice(i * BPC, (i + 1) * BPC)
        qpc = qT[sl].reshape(NGRP, NB, 128, DC, Lq).transpose(0, 2, 1, 3, 4)
        mixc = mix[sl].reshape(NGRP, NB, 128, DC, 164).transpose(0, 2, 1, 3, 4)
        cnc = cN[sl].reshape(NGRP, NB, Ls, D)
        cn0c = cnc[:, 0:2].transpose(0, 2, 1, 3)  # [NGRP, 36, 2, D]
        cn1c = cnc[:, 2:4].transpose(0, 2, 1, 3)
        in_maps.append(
            {
                "qp": np.ascontiguousarray(qpc),
                "mixp": np.ascontiguousarray(mixc),
                "cn0": np.ascontiguousarray(cn0c),
                "cn1": np.ascontiguousarray(cn1c),
                "wT": wTh,
            }
        )
    return in_maps


def _run(query, context, matrix, smooth, W, b, trace=False, opts=None):
    from concourse.bass_utils import run_bass_kernel_spmd

    smooth_f = float(smooth)
    key = (smooth_f, str(sorted((opts or {}).items())))
    if key not in _CACHE:
        _CACHE[key] = _build(smooth_f, opts)
    nc = _CACHE[key]

    in_maps = _prep_inputs(query, context, matrix, smooth_f, W, b)
    res = run_bass_kernel_spmd(nc, in_maps, core_ids=list(range(N_CORES)), trace=trace)
    parts = []
    for i in range(N_CORES):
        o = np.asarray(res.results[i]["out"], dtype=np.float32)
        parts.append(o.transpose(0, 2, 1, 3).reshape(BPC, Lq, S))
    return np.concatenate(parts, axis=0), res


def kernel(query, context, matrix, smooth, W, b):
    query = np.asarray(query, dtype=np.float32)
    context = np.asarray(context, dtype=np.float32)
    matrix = np.asarray(matrix, dtype=np.float32)
    W = np.asarray(W, dtype=np.float32)
    b = np.asarray(b, dtype=np.float32)
    out, _ = _run(query, context, matrix, smooth, W, b, trace=False)
    return out


def kernel_profiled(query, context, matrix, smooth, W, b, reps=3, opts=None):
    out, res = _run(query, context, matrix, smooth, W, b, trace=True, opts=opts)
    times = [res.exec_time_ns]
    for _ in range(reps - 1):
        _, r2 = _run(query, context, matrix, smooth, W, b, trace=True, opts=opts)
        times.append(r2.exec_time_ns)
    res.all_times = times
    return out, res
